# revision 41
# baseline (speedup 1.0000x reference)
"""Trainium2 Bass kernel for nn_MesoNet (gnn_message_passing).

Strategy: 8-way data-parallel sharding. Nodes are sharded into 8 contiguous
blocks of 256; edges are sharded by the block that owns their dst node, so
every scatter is core-local. Each core runs the full NNConv
gather-einsum-scatter pipeline on its edge shard in bf16:

  h   = relu(edge_attr @ w1 + b1)                  (PE + ACT)
  Q   = x[src] @ W2P                               (PE, W2P = w2 reordered
                                                    host-side to [din, dout*33],
                                                    33rd f-slot carries b2)
  QH  = Q * h'                                     (DVE stt, h' = [h, 1])
  msg = reduce_f QH                                (DVE reduce)
  s^T = msg^T @ A_w + root^T @ x^T                 (PE, A_w = one-hot(dst)
                                                    pre-scaled by 1/deg -> fused
                                                    mean aggregation)
  out = relu(s^T + bias)                           (ACT, per-partition bias)

Node features live transposed ([feat, node]) in SBUF; feature tables for the
next layer's gathers are rebuilt (PE transpose) and AllGathered across cores.
The [64, B] pooled features are AllReduce-summed (P_w one-hot pre-scaled by
1/graph-count) before a replicated MLP head.
"""

import sys

for _p in ("/opt/trn_rl_repo",):
    if _p not in sys.path:
        sys.path.insert(0, _p)

import numpy as np

import concourse.bass as bass
import concourse.bacc as bacc
import concourse.mybir as mybir
import concourse.tile as tile
from concourse.masks import make_identity

bf16 = mybir.dt.bfloat16
f32 = mybir.dt.float32
i32 = mybir.dt.int32
AF = mybir.ActivationFunctionType
OP = mybir.AluOpType

N, E, B = 2048, 4096, 64
NCORES = 8
NODES_C = N // NCORES          # 256
E_PAD = 640                    # per-core edge budget (5 tiles of 128)
ET = E_PAD // 128              # edge tiles per core
FD, HD, EDGE_DIM = 32, 160, 12
FP = FD + 1                    # virtual f slot for b2

CONVS = {
    "a11": dict(din=41, dout=32),
    "a21": dict(din=3 * FD, dout=3 * FD),
    "sc1": dict(din=HD, dout=HD),
    "sc2": dict(din=HD, dout=HD),
}


def _ochunks(dout, width=30):
    """Split output-channel range into chunks; chunk*33 cols per Q matmul."""
    out = []
    o = 0
    while o < dout:
        out.append((o, min(o + width, dout)))
        o += width
    return out


def _kchunks(din):
    out = []
    k = 0
    while k < din:
        out.append((k, min(k + 128, din)))
        k += 128
    return out


def build_program():
    nc = bacc.Bacc("TRN2", target_bir_lowering=False, debug=False,
                   num_devices=NCORES)

    def inp(name, shape, dt):
        return nc.dram_tensor(name, shape, dt, kind="ExternalInput").ap()

    t_xT41 = inp("xT41", [41, NODES_C], bf16)
    t_xTmid = inp("xTmid", [6, NODES_C], bf16)
    t_xTg = inp("xTg", [55, NODES_C], bf16)
    t_x41 = inp("x41t", [N, 41], bf16)
    t_eaT = inp("eaT", [13, E_PAD], bf16)
    t_src = inp("srcidx", [E_PAD, 1], i32)
    t_dst = inp("dstf", [E_PAD, 1], f32)
    t_we = inp("wef", [E_PAD, 1], f32)
    t_batch = inp("batchf", [NODES_C, 1], f32)
    t_wpool = inp("wpool", [NODES_C, 1], f32)

    wt = {}
    for L, cfg in CONVS.items():
        din, dout = cfg["din"], cfg["dout"]
        wt[f"{L}_w1"] = inp(f"{L}_w1", [13, FD], bf16)
        wt[f"{L}_w2p"] = inp(f"{L}_w2p", [din, dout * FP], bf16)
        wt[f"{L}_root"] = inp(f"{L}_root", [din, dout], bf16)
        wt[f"{L}_b"] = inp(f"{L}_b", [dout, 1], f32)
    for nm, sh in [("x2_w", [6, FD]), ("inter_wv", [55, FD]),
                   ("inter_wo", [FD, FD]), ("gg_wv", [FD, FD]),
                   ("gg_wo", [FD, FD]), ("ga_wv", [FD, FD]),
                   ("ga_wo", [FD, FD]), ("cfc_wg", [96, 64]),
                   ("cfc_wh", [96, 64]), ("cfc_wt", [96, 64]),
                   ("cfc_wout", [64, FD]), ("x22_w", [96, 96]),
                   ("xm3_w", [HD, HD]), ("fc1_w", [HD, 128]),
                   ("fc2_w", [128, 1])]:
        wt[nm] = inp(nm, sh, bf16)
    for nm, d in [("x2_b", FD), ("cfc_bg", 64), ("cfc_bh", 64),
                  ("cfc_bt", 64), ("x22_b", 96), ("xm3_b", HD),
                  ("fc1_b", 128), ("fc2_b", 1)]:
        wt[nm] = inp(nm, [d, 1], f32)

    t_out = nc.dram_tensor("out", [B, 1], f32, kind="ExternalOutput").ap()

    RG = [list(range(NCORES))]

    with tile.TileContext(nc) as tc:
        import contextlib
        ctx = contextlib.ExitStack()
        with ctx:
            sbw = ctx.enter_context(tc.tile_pool(name="sbw", bufs=1))
            sbf = ctx.enter_context(tc.tile_pool(name="sbf", bufs=1))
            sbk = ctx.enter_context(tc.tile_pool(name="sbk", bufs=3))
            psq = ctx.enter_context(tc.tile_pool(name="psq", bufs=2, space="PSUM"))
            pss = ctx.enter_context(tc.tile_pool(name="pss", bufs=1, space="PSUM"))
            psm = ctx.enter_context(tc.tile_pool(name="psm", bufs=2, space="PSUM"))
            dram = ctx.enter_context(tc.tile_pool(name="dram", bufs=1, space="DRAM"))

            # ---------- constants & weight staging ----------
            ident = sbw.tile([128, 128], bf16)
            make_identity(nc, ident[:])
            iota_i = sbw.tile([128, NODES_C], i32)
            nc.gpsimd.iota(iota_i[:], pattern=[[1, NODES_C]], base=0,
                           channel_multiplier=0)
            iota_b = sbw.tile([128, NODES_C], bf16)
            nc.vector.tensor_copy(iota_b[:], iota_i[:])

            # weight row-splits: matmul operands must start at a fresh tile
            # (base partition 0), so split every weight at the row boundaries
            # its matmuls contract over.
            SPLITS = {"cfc_wg": [32, 64], "cfc_wh": [32, 64],
                      "cfc_wt": [32, 64], "cfc_wout": [32],
                      "x22_w": [32, 64], "xm3_w": [96, 128]}
            W = {}
            for nm, ap in wt.items():
                p, f = ap.shape
                cuts = SPLITS.get(nm, [128] if p > 128 else [])
                bounds = [0] + list(cuts) + [p]
                tiles = []
                for bi in range(len(bounds) - 1):
                    r0, r1 = bounds[bi], bounds[bi + 1]
                    tw = sbw.tile([r1 - r0, f], ap.dtype, name=f"w_{nm}_{bi}")
                    nc.sync.dma_start(tw[:], ap[r0:r1, :])
                    tiles.append(tw)
                W[nm] = tiles

            eaT = sbw.tile([13, E_PAD], bf16)
            nc.sync.dma_start(eaT[:], t_eaT)

            # per-edge-tile index/scatter tiles
            idxt, awt = [], []
            for t in range(ET):
                sl = slice(t * 128, (t + 1) * 128)
                it = sbw.tile([128, 1], i32, name=f"idx{t}")
                nc.sync.dma_start(it[:], t_src[sl, :])
                idxt.append(it)
                dt_ = sbw.tile([128, 1], f32, name=f"dst{t}")
                nc.sync.dma_start(dt_[:], t_dst[sl, :])
                we_ = sbw.tile([128, 1], f32, name=f"we{t}")
                nc.sync.dma_start(we_[:], t_we[sl, :])
                eq = sbk.tile([128, NODES_C], bf16, tag="eqtmp")
                nc.vector.tensor_tensor(
                    eq[:], iota_b[:], dt_[:, 0:1].to_broadcast([128, NODES_C]),
                    op=OP.is_equal)
                aw = sbw.tile([128, NODES_C], bf16, name=f"aw{t}")
                nc.vector.tensor_tensor(
                    aw[:], eq[:], we_[:, 0:1].to_broadcast([128, NODES_C]),
                    op=OP.mult)
                awt.append(aw)

            # pooling one-hot (scaled)
            pwt = []
            for t in range(2):
                sl = slice(t * 128, (t + 1) * 128)
                bt = sbw.tile([128, 1], f32, name=f"bat{t}")
                nc.sync.dma_start(bt[:], t_batch[sl, :])
                wp = sbw.tile([128, 1], f32, name=f"wp{t}")
                nc.sync.dma_start(wp[:], t_wpool[sl, :])
                eqp = sbk.tile([128, B], bf16, tag="eqtmp2")
                nc.vector.tensor_tensor(
                    eqp[:], iota_b[:, 0:B], bt[:, 0:1].to_broadcast([128, B]),
                    op=OP.is_equal)
                pw = sbw.tile([128, B], bf16, name=f"pw{t}")
                nc.vector.tensor_tensor(
                    pw[:], eqp[:], wp[:, 0:1].to_broadcast([128, B]),
                    op=OP.mult)
                pwt.append(pw)

            xT41 = sbw.tile([41, NODES_C], bf16)
            nc.sync.dma_start(xT41[:], t_xT41)
            xTmid = sbw.tile([6, NODES_C], bf16)
            nc.sync.dma_start(xTmid[:], t_xTmid)
            xTg = sbw.tile([55, NODES_C], bf16)
            nc.sync.dma_start(xTg[:], t_xTg)

            # ---------- helpers ----------
            def mm_chain(psum_ap, parts, stop=True):
                """parts: list of (lhsT_ap, rhs_ap); accumulate into psum_ap."""
                n = len(parts)
                for i, (lh, rh) in enumerate(parts):
                    nc.tensor.matmul(out=psum_ap, lhsT=lh, rhs=rh,
                                     start=(i == 0), stop=(stop and i == n - 1))

            def act_evict(out_ap, psum_ap, func, bias=0.0):
                nc.scalar.activation(out_ap, psum_ap, func, bias=bias)

            def lin_T(rhs_parts, wname, M, func=AF.Copy, bias=None, name="lt"):
                """out[M, NODES_C] = func(sum_k w[k-chunk].T @ rhs_parts[k] + b).

                rhs_parts: list of SBUF tiles covering the contraction dim in
                chunks matching W[wname] row tiles; M = out channels (<=128).
                """
                ps = psm.tile([M, NODES_C], f32, name=f"ps_{name}", tag="mps")
                parts = []
                wtl = W[wname]
                assert len(wtl) == len(rhs_parts)
                for wti, rp in zip(wtl, rhs_parts):
                    parts.append((wti[:, 0:M], rp))
                mm_chain(ps[:], parts)
                ot = sbf.tile([M, NODES_C], bf16, name=name)
                if bias is not None:
                    act_evict(ot[:], ps[:], func, bias=bias)
                else:
                    act_evict(ot[:], ps[:], func)
                return ot

            def transpose_to(out_tile, out_col0, in_ap, w, name):
                """PE-transpose in_ap [p<=128, w<=128] -> out_tile[:, col0:col0+p]"""
                p = in_ap.shape[0]
                tp = psm.tile([w, 128], bf16, space="PSUM", name=f"tp_{name}",
                              tag="mps")
                nc.tensor.transpose(out=tp[:w, 0:p], in_=in_ap,
                                    identity=ident[0:p, 0:p])
                nc.vector.tensor_copy(out_tile[:, out_col0:out_col0 + p],
                                      tp[:w, 0:p])

            # ---------- front end (node-dense, transposed) ----------
            x2f = lin_T([xTmid[:]], "x2_w", FD, AF.Relu,
                        W["x2_b"][0][:, 0:1], name="x2f")
            p1 = lin_T([xTg[:]], "inter_wv", FD, name="p1")
            interT = lin_T([p1[:]], "inter_wo", FD, name="interT")
            p2 = lin_T([interT[:]], "gg_wv", FD, name="p2")
            globT = lin_T([p2[:]], "gg_wo", FD, name="globT")
            p3 = lin_T([globT[:]], "ga_wv", FD, name="p3")
            grpT = lin_T([p3[:]], "ga_wo", FD, name="grpT")

            # CfC, 3 unrolled steps; h kept as two [32, NODES_C] tiles
            uT = x2f
            h_lo, h_hi = grpT, globT
            steps = []
            for st in range(3):
                gates = {}
                for gname, wn, bn, fn in [("g1", "cfc_wg", "cfc_bg", AF.Tanh),
                                          ("g2", "cfc_wh", "cfc_bh", AF.Tanh),
                                          ("tt", "cfc_wt", "cfc_bt", AF.Sigmoid)]:
                    wk = W[wn]
                    btile = W[bn][0]
                    halves = []
                    for hf in range(2):
                        cs = slice(hf * FD, hf * FD + FD)
                        ps = psm.tile([FD, NODES_C], f32,
                                      name=f"cfc{st}{gname}{hf}", tag="mps")
                        mm_chain(ps[:], [(wk[0][:, cs], uT[:]),
                                         (wk[1][:, cs], h_lo[:]),
                                         (wk[2][:, cs], h_hi[:])])
                        g = sbf.tile([FD, NODES_C], bf16,
                                     name=f"{gname}{st}{hf}")
                        act_evict(g[:], ps[:], fn, bias=btile[cs, 0:1])
                        halves.append(g)
                    gates[gname] = halves
                new_h = []
                for hf in range(2):
                    g1, g2, tg = gates["g1"][hf], gates["g2"][hf], gates["tt"][hf]
                    d = sbk.tile([FD, NODES_C], bf16, tag="cfc_d")
                    nc.vector.tensor_tensor(d[:], g2[:], g1[:], op=OP.subtract)
                    td = sbk.tile([FD, NODES_C], bf16, tag="cfc_td")
                    nc.vector.tensor_tensor(td[:], tg[:], d[:], op=OP.mult)
                    hn = sbf.tile([FD, NODES_C], bf16, name=f"h{st}{hf}")
                    nc.vector.tensor_tensor(hn[:], g1[:], td[:], op=OP.add)
                    new_h.append(hn)
                h_lo, h_hi = new_h
                psu = psm.tile([FD, NODES_C], f32, name=f"cfcu{st}", tag="mps")
                mm_chain(psu[:], [(W["cfc_wout"][0][:], h_lo[:]),
                                  (W["cfc_wout"][1][:], h_hi[:])])
                uT = sbf.tile([FD, NODES_C], bf16, name=f"u{st}")
                act_evict(uT[:], psu[:], AF.Copy)
                steps.append(uT)

            # x2o = relu(cat(steps) @ x22_w + b)   [96, NODES_C]
            ps_x2o = psm.tile([96, NODES_C], f32, tag="mps")
            mm_chain(ps_x2o[:], [(W["x22_w"][k][:], steps[k][:])
                                 for k in range(3)])
            x2o = sbf.tile([96, NODES_C], bf16)
            act_evict(x2o[:], ps_x2o[:], AF.Relu, bias=W["x22_b"][0][:, 0:1])

            # ---------- conv layer runner ----------
            def conv_layer(L, table_ap, featT_parts, name):
                """Run NNConv L. table_ap: DRAM [N, din] bf16 row table.
                featT_parts: transposed own-node features (k-chunk tiles).
                Returns list of out tiles ([<=128, NODES_C]) covering dout."""
                din, dout = CONVS[L]["din"], CONVS[L]["dout"]
                kcs = _kchunks(din)
                ocs = _ochunks(dout)
                w2p = W[f"{L}_w2p"]
                w1 = W[f"{L}_w1"][0]
                root = W[f"{L}_root"]
                bias_tiles = W[f"{L}_b"]
                mcs = _kchunks(dout)  # out partition chunks

                s_ps = [pss.tile([m1 - m0, NODES_C], f32,
                                 name=f"sps_{name}{mi}", tag=f"sps{mi}")
                        for mi, (m0, m1) in enumerate(mcs)]

                n_acc = ET + len(kcs)  # matmuls accumulated per psum
                acc_i = [0] * len(mcs)

                for t in range(ET):
                    esl = slice(t * 128, (t + 1) * 128)
                    # edge MLP h
                    h_ps = psm.tile([128, FD], f32, tag="mps")
                    nc.tensor.matmul(out=h_ps[:], lhsT=eaT[:, esl], rhs=w1[:],
                                     start=True, stop=True)
                    hp = sbk.tile([128, FP], bf16, tag="hp")
                    act_evict(hp[:, 0:FD], h_ps[:], AF.Relu)
                    nc.vector.memset(hp[:, FD:FP], 1.0)
                    # gather + transpose
                    xr = sbk.tile([128, din], bf16, tag="xr")
                    nc.gpsimd.indirect_dma_start(
                        out=xr[:], out_offset=None, in_=table_ap,
                        in_offset=bass.IndirectOffsetOnAxis(
                            ap=idxt[t][:, 0:1], axis=0))
                    xsT = []
                    for ki, (k0, k1) in enumerate(kcs):
                        xt = sbk.tile([k1 - k0, 128], bf16, tag=f"xsT{ki}")
                        tp = psm.tile([k1 - k0, 128], bf16, space="PSUM",
                                      tag="mps")
                        nc.tensor.transpose(out=tp[:], in_=xr[:, k0:k1],
                                            identity=ident[:])
                        nc.vector.tensor_copy(xt[:], tp[:])
                        xsT.append(xt)
                    # Q chunks -> QH -> msg. Each o-chunk (<=30 o's) maps to
                    # two bank-aligned matmul windows of one 2-bank PSUM tile
                    # (a single matmul output may not cross a PSUM bank).
                    msg = sbk.tile([128, dout], bf16, tag="msg")
                    for (o0, o1) in ocs:
                        ow = o1 - o0
                        wins = [(0, min(15, ow))]
                        if ow > 15:
                            wins.append((15, ow))
                        q_ps = psq.tile([128, 1024], f32, tag="qps")
                        for wi, (a, b) in enumerate(wins):
                            for ki in range(len(kcs)):
                                nc.tensor.matmul(
                                    out=q_ps[:, wi * 512:wi * 512 + (b - a) * FP],
                                    lhsT=xsT[ki][:],
                                    rhs=w2p[ki][:, (o0 + a) * FP:(o0 + b) * FP],
                                    start=(ki == 0), stop=(ki == len(kcs) - 1))
                        qh = sbk.tile([128, ow * FP], bf16, tag="qh")
                        if ow == 30:
                            # one TT over both windows via a gapped 4D view
                            nc.vector.tensor_tensor(
                                qh[:].rearrange("p (b o f) -> p b o f",
                                                b=2, f=FP),
                                q_ps[:].rearrange("p (b x) -> p b x", b=2)
                                [:, :, 0:15 * FP].rearrange(
                                    "p b (o f) -> p b o f", f=FP),
                                hp[:, None, None, 0:FP].to_broadcast(
                                    [128, 2, 15, FP]),
                                op=OP.mult)
                        else:
                            for wi, (a, b) in enumerate(wins):
                                nc.vector.tensor_tensor(
                                    qh[:, a * FP:b * FP].rearrange(
                                        "p (o f) -> p o f", f=FP),
                                    q_ps[:, wi * 512:wi * 512 + (b - a) * FP]
                                    .rearrange("p (o f) -> p o f", f=FP),
                                    hp[:, None, 0:FP].to_broadcast(
                                        [128, b - a, FP]),
                                    op=OP.mult)
                        with nc.allow_low_precision("bf16 msg accum"):
                            nc.vector.tensor_reduce(
                                out=msg[:, o0:o1],
                                in_=qh[:].rearrange("p (o f) -> p o f", f=FP),
                                axis=mybir.AxisListType.X, op=OP.add)
                    # scatter
                    for mi, (m0, m1) in enumerate(mcs):
                        nc.tensor.matmul(out=s_ps[mi][:],
                                         lhsT=msg[:, m0:m1], rhs=awt[t][:],
                                         start=(acc_i[mi] == 0), stop=False)
                        acc_i[mi] += 1
                # root term
                outs = []
                for mi, (m0, m1) in enumerate(mcs):
                    for ki, (k0, k1) in enumerate(kcs):
                        acc_i[mi] += 1
                        nc.tensor.matmul(out=s_ps[mi][:],
                                         lhsT=root[ki][:, m0:m1],
                                         rhs=featT_parts[ki][:],
                                         start=False,
                                         stop=(acc_i[mi] == n_acc))
                    ot = sbf.tile([m1 - m0, NODES_C], bf16,
                                  name=f"conv_{name}{mi}")
                    act_evict(ot[:], s_ps[mi][:], AF.Relu,
                              bias=bias_tiles[mi][:, 0:1])
                    outs.append(ot)
                return outs

            def rows_and_allgather(featT_parts, D, name):
                """Transpose [feat,node] tiles into row tiles, DMA to DRAM,
                AllGather -> full [N, D] table."""
                slab = dram.tile([NODES_C, D], bf16, name=f"slab_{name}")
                tbl = dram.tile([N, D], bf16, addr_space="Shared",
                                name=f"tbl_{name}")
                for ntile in range(2):
                    nsl = slice(ntile * 128, (ntile + 1) * 128)
                    rows = sbk.tile([128, D], bf16, tag="rows")
                    col0 = 0
                    for ki, ft in enumerate(featT_parts):
                        w = ft.shape[0]
                        transpose_to(rows, col0, ft[:, nsl], 128,
                                     f"r{name}{ntile}{ki}")
                        col0 += w
                    nc.sync.dma_start(slab[nsl, :], rows[:])
                nc.gpsimd.collective_compute(
                    "AllGather", OP.bypass, ins=[slab.opt()], outs=[tbl.opt()],
                    replica_groups=RG)
                return tbl

            # ---------- a11 ----------
            x1 = conv_layer("a11", t_x41, [xT41[:]], "a11")[0]

            # ---------- a21 ----------
            tbl_a21 = rows_and_allgather([x2o], 96, "a21")
            x2o2 = conv_layer("a21", tbl_a21[:], [x2o[:]], "a21")[0]

            # ---------- xm ----------
            xm_parts = []
            for mi, (m0, m1) in enumerate(_kchunks(HD)):
                ps = psm.tile([m1 - m0, NODES_C], f32, name=f"psxm{mi}",
                              tag="mps")
                mm_chain(ps[:], [(W["xm3_w"][0][:, m0:m1], x2o2[:]),
                                 (W["xm3_w"][1][:, m0:m1], x1[:]),
                                 (W["xm3_w"][2][:, m0:m1], globT[:])])
                xm = sbf.tile([m1 - m0, NODES_C], bf16, name=f"xm{mi}")
                act_evict(xm[:], ps[:], AF.Relu,
                          bias=W["xm3_b"][mi][:, 0:1])
                xm_parts.append(xm)

            # ---------- sc1, sc2 ----------
            tbl_sc1 = rows_and_allgather(xm_parts, HD, "sc1")
            s1_parts = conv_layer("sc1", tbl_sc1[:], xm_parts, "sc1")
            tbl_sc2 = rows_and_allgather(s1_parts, HD, "sc2")
            s2_parts = conv_layer("sc2", tbl_sc2[:], s1_parts, "sc2")

            # ---------- pooling ----------
            pl_ps = pss.tile([B, HD], f32, tag="sps0")
            acc = 0
            for ntile in range(2):
                nsl = slice(ntile * 128, (ntile + 1) * 128)
                s2r = sbk.tile([128, HD], bf16, tag="rows")
                col0 = 0
                for ki, ft in enumerate(s2_parts):
                    w = ft.shape[0]
                    transpose_to(s2r, col0, ft[:, nsl], 128, f"pool{ntile}{ki}")
                    col0 += w
                nc.tensor.matmul(out=pl_ps[:], lhsT=pwt[ntile][:], rhs=s2r[:],
                                 start=(acc == 0), stop=(acc == 1))
                acc += 1
            pl_sb = sbf.tile([B, HD], f32)
            nc.vector.tensor_copy(pl_sb[:], pl_ps[:])
            ar_in = dram.tile([B, HD], f32)
            ar_out = dram.tile([B, HD], f32, addr_space="Shared")
            nc.sync.dma_start(ar_in[:], pl_sb[:])
            nc.gpsimd.collective_compute(
                "AllReduce", OP.add, ins=[ar_in.opt()], outs=[ar_out.opt()],
                replica_groups=RG)
            pooled = sbf.tile([B, HD], f32)
            nc.sync.dma_start(pooled[:], ar_out[:])

            # ---------- head ----------
            pooled_bf = sbf.tile([B, HD], bf16)
            nc.vector.tensor_copy(pooled_bf[:], pooled[:])
            pT_parts = []
            for ki, (k0, k1) in enumerate(_kchunks(HD)):
                tp = psm.tile([k1 - k0, B], bf16, space="PSUM",
                              name=f"ptp{ki}", tag="mps")
                nc.tensor.transpose(out=tp[:], in_=pooled_bf[:, k0:k1],
                                    identity=ident[0:B, 0:B])
                pt = sbf.tile([k1 - k0, B], bf16, name=f"pT{ki}")
                nc.vector.tensor_copy(pt[:], tp[:])
                pT_parts.append(pt)
            ps_t1 = psm.tile([128, B], f32, tag="mps")
            mm_chain(ps_t1[:], [(W["fc1_w"][0][:, :], pT_parts[0][:]),
                                (W["fc1_w"][1][:, :], pT_parts[1][:])])
            t1 = sbf.tile([128, B], bf16)
            act_evict(t1[:], ps_t1[:], AF.Relu, bias=W["fc1_b"][0][:, 0:1])
            ps_o = psm.tile([1, B], f32, tag="mps")
            nc.tensor.matmul(out=ps_o[:], lhsT=W["fc2_w"][0][:, :], rhs=t1[:],
                             start=True, stop=True)
            orow = sbf.tile([1, B], f32)
            act_evict(orow[:], ps_o[:], AF.Identity, bias=W["fc2_b"][0][:, 0:1])
            nc.sync.dma_start(t_out.rearrange("a b -> b a"), orow[:])

    nc.compile()
    return nc


_NC_CACHE = {}


def _get_program():
    if "nc" not in _NC_CACHE:
        _NC_CACHE["nc"] = build_program()
    return _NC_CACHE["nc"]


def _bf(x):
    import ml_dtypes
    return np.asarray(x, np.float32).astype(ml_dtypes.bfloat16)


def prepare_in_maps(x, edge_index, edge_attr, batch, params):
    x = np.asarray(x, np.float32)
    edge_index = np.asarray(edge_index, np.int64)
    edge_attr = np.asarray(edge_attr, np.float32)
    batch = np.asarray(batch, np.int64)
    p = {k: np.asarray(v, np.float32) for k, v in params.items()
         if not isinstance(v, dict)}
    for k, v in params.items():
        if isinstance(v, dict):
            p[k] = {kk: np.asarray(vv, np.float32) for kk, vv in v.items()}

    src, dst = edge_index[0], edge_index[1]
    cnt = np.bincount(dst, minlength=N).astype(np.float32)
    w_edge_all = 1.0 / np.maximum(cnt, 1.0)
    pcnt = np.bincount(batch, minlength=B).astype(np.float32)
    wpool_all = 1.0 / np.maximum(pcnt, 1.0)

    # shared (replicated) weight arrays
    shared = {"x41t": _bf(x[:, :41])}
    for L in CONVS:
        cfg = CONVS[L]
        din, dout = cfg["din"], cfg["dout"]
        pl = p[L]
        w1a = np.concatenate([pl["w1"], pl["b1"][None, :]], 0)  # [13, 32]
        shared[f"{L}_w1"] = _bf(w1a)
        w2r = pl["w2"].reshape(FD, din, dout)
        b2 = pl["b2"].reshape(din, dout)
        w2p = np.concatenate([w2r.transpose(1, 2, 0), b2[:, :, None]],
                             axis=2).reshape(din, dout * FP)
        shared[f"{L}_w2p"] = _bf(w2p)
        shared[f"{L}_root"] = _bf(pl["root"])
        shared[f"{L}_b"] = pl["bias"].reshape(dout, 1).astype(np.float32)
    for nm in ["x2_w", "inter_wv", "inter_wo", "gg_wv", "gg_wo", "ga_wv",
               "ga_wo", "cfc_wg", "cfc_wh", "cfc_wt", "cfc_wout", "x22_w",
               "xm3_w", "fc1_w", "fc2_w"]:
        shared[nm] = _bf(p[nm])
    for nm in ["x2_b", "cfc_bg", "cfc_bh", "cfc_bt", "x22_b", "xm3_b",
               "fc1_b", "fc2_b"]:
        shared[nm] = p[nm].reshape(-1, 1).astype(np.float32)

    in_maps = []
    owner = dst // NODES_C
    for c in range(NCORES):
        sel = np.nonzero(owner == c)[0]
        ec = len(sel)
        assert ec <= E_PAD, f"core {c} has {ec} edges > E_PAD={E_PAD}"
        srcc = np.zeros(E_PAD, np.int32)
        dstc = np.zeros(E_PAD, np.float32)
        wec = np.zeros(E_PAD, np.float32)
        eac = np.zeros((13, E_PAD), np.float32)
        srcc[:ec] = src[sel]
        dstc[:ec] = (dst[sel] - c * NODES_C).astype(np.float32)
        wec[:ec] = w_edge_all[dst[sel]]
        eac[:12, :ec] = edge_attr[sel].T
        eac[12, :ec] = 1.0
        nsl = slice(c * NODES_C, (c + 1) * NODES_C)
        m = dict(shared)
        xcT = x[nsl].T
        m["xT41"] = _bf(xcT[0:41])
        m["xTmid"] = _bf(xcT[41:47])
        m["xTg"] = _bf(xcT[160:215])
        m["eaT"] = _bf(eac)
        m["srcidx"] = srcc.reshape(E_PAD, 1)
        m["dstf"] = dstc.reshape(E_PAD, 1)
        m["wef"] = wec.reshape(E_PAD, 1)
        m["batchf"] = batch[nsl].astype(np.float32).reshape(NODES_C, 1)
        m["wpool"] = wpool_all[batch[nsl]].astype(np.float32).reshape(
            NODES_C, 1)
        in_maps.append(m)
    return in_maps


def kernel(x, edge_index, edge_attr, batch, params):
    in_maps = prepare_in_maps(x, edge_index, edge_attr, batch, params)
    from concourse.bass_utils import run_bass_kernel_spmd
    nc = _get_program()
    res = run_bass_kernel_spmd(nc, in_maps, core_ids=list(range(NCORES)))
    return np.asarray(res.results[0]["out"], np.float32)


if __name__ == "__main__":
    sys.path.insert(0, "/root/problem")
    import reference
    inputs = {k: np.asarray(v) for k, v in reference.setup_inputs().items()
              if not isinstance(v, dict)}
    inputs["params"] = {
        k: (np.asarray(v) if not isinstance(v, dict)
            else {kk: np.asarray(vv) for kk, vv in v.items()})
        for k, v in reference.setup_inputs()["params"].items()}
    out = kernel(**inputs)
    print(out[:8, 0])


# revision 44
# speedup vs baseline: 1.3078x; 1.3078x over previous
"""Trainium2 Bass kernel for nn_MesoNet (gnn_message_passing).

Strategy: 8-way data-parallel sharding. Nodes are sharded into 8 contiguous
blocks of 256; edges are sharded by the block that owns their dst node, so
every scatter is core-local. Each core runs the full NNConv
gather-einsum-scatter pipeline on its edge shard in bf16:

  h   = relu(edge_attr @ w1 + b1)                  (PE + ACT)
  Q   = x[src] @ W2P                               (PE, W2P = w2 reordered
                                                    host-side to [din, dout*33],
                                                    33rd f-slot carries b2)
  QH  = Q * h'                                     (DVE stt, h' = [h, 1])
  msg = reduce_f QH                                (DVE reduce)
  s^T = msg^T @ A_w + root^T @ x^T                 (PE, A_w = one-hot(dst)
                                                    pre-scaled by 1/deg -> fused
                                                    mean aggregation)
  out = relu(s^T + bias)                           (ACT, per-partition bias)

Node features live transposed ([feat, node]) in SBUF; feature tables for the
next layer's gathers are rebuilt (PE transpose) and AllGathered across cores.
The [64, B] pooled features are AllReduce-summed (P_w one-hot pre-scaled by
1/graph-count) before a replicated MLP head.
"""

import sys

for _p in ("/opt/trn_rl_repo",):
    if _p not in sys.path:
        sys.path.insert(0, _p)

import numpy as np

import concourse.bass as bass
import concourse.bacc as bacc
import concourse.mybir as mybir
import concourse.tile as tile
from concourse.masks import make_identity

bf16 = mybir.dt.bfloat16
f32 = mybir.dt.float32
i32 = mybir.dt.int32
AF = mybir.ActivationFunctionType
OP = mybir.AluOpType

N, E, B = 2048, 4096, 64
NCORES = 8
NODES_C = N // NCORES          # 256
E_PAD = 640                    # per-core edge budget (5 tiles of 128)
ET = E_PAD // 128              # edge tiles per core
FD, HD, EDGE_DIM = 32, 160, 12
FP = FD + 1                    # virtual f slot for b2

CONVS = {
    "a11": dict(din=41, dout=32),
    "a21": dict(din=3 * FD, dout=3 * FD),
    "sc1": dict(din=HD, dout=HD),
    "sc2": dict(din=HD, dout=HD),
}


def _ochunks(dout, width=15):
    """Split output-channel range into chunks; chunk*33 cols per Q matmul."""
    out = []
    o = 0
    while o < dout:
        out.append((o, min(o + width, dout)))
        o += width
    return out


def _kchunks(din):
    out = []
    k = 0
    while k < din:
        out.append((k, min(k + 128, din)))
        k += 128
    return out


def build_program():
    nc = bacc.Bacc("TRN2", target_bir_lowering=False, debug=False,
                   num_devices=NCORES)

    def inp(name, shape, dt):
        return nc.dram_tensor(name, shape, dt, kind="ExternalInput").ap()

    t_xT41 = inp("xT41", [41, NODES_C], bf16)
    t_xTmid = inp("xTmid", [6, NODES_C], bf16)
    t_xTg = inp("xTg", [55, NODES_C], bf16)
    t_x41 = inp("x41t", [N, 41], bf16)
    t_eaT = inp("eaT", [13, E_PAD], bf16)
    t_src = inp("srcidx", [E_PAD, 1], i32)
    t_dst = inp("dstf", [E_PAD, 1], f32)
    t_we = inp("wef", [E_PAD, 1], f32)
    t_batch = inp("batchf", [NODES_C, 1], f32)
    t_wpool = inp("wpool", [NODES_C, 1], f32)

    wt = {}
    for L, cfg in CONVS.items():
        din, dout = cfg["din"], cfg["dout"]
        wt[f"{L}_w1"] = inp(f"{L}_w1", [13, FD], bf16)
        wt[f"{L}_w2p"] = inp(f"{L}_w2p", [din, dout * FP], bf16)
        wt[f"{L}_root"] = inp(f"{L}_root", [din, dout], bf16)
        wt[f"{L}_b"] = inp(f"{L}_b", [dout, 1], f32)
    for nm, sh in [("x2_w", [6, FD]), ("inter_wv", [55, FD]),
                   ("inter_wo", [FD, FD]), ("gg_wv", [FD, FD]),
                   ("gg_wo", [FD, FD]), ("ga_wv", [FD, FD]),
                   ("ga_wo", [FD, FD]), ("cfc_wg", [96, 64]),
                   ("cfc_wh", [96, 64]), ("cfc_wt", [96, 64]),
                   ("cfc_wout", [64, FD]), ("x22_w", [96, 96]),
                   ("xm3_w", [HD, HD]), ("fc1_w", [HD, 128]),
                   ("fc2_w", [128, 1])]:
        wt[nm] = inp(nm, sh, bf16)
    for nm, d in [("x2_b", FD), ("cfc_bg", 64), ("cfc_bh", 64),
                  ("cfc_bt", 64), ("x22_b", 96), ("xm3_b", HD),
                  ("fc1_b", 128), ("fc2_b", 1)]:
        wt[nm] = inp(nm, [d, 1], f32)

    t_out = nc.dram_tensor("out", [B, 1], f32, kind="ExternalOutput").ap()

    RG = [list(range(NCORES))]

    with tile.TileContext(nc) as tc:
        import contextlib
        ctx = contextlib.ExitStack()
        with ctx:
            sbw = ctx.enter_context(tc.tile_pool(name="sbw", bufs=1))
            sbf = ctx.enter_context(tc.tile_pool(name="sbf", bufs=1))
            sbk = ctx.enter_context(tc.tile_pool(name="sbk", bufs=3))
            psq = ctx.enter_context(tc.tile_pool(name="psq", bufs=2, space="PSUM"))
            pss = ctx.enter_context(tc.tile_pool(name="pss", bufs=1, space="PSUM"))
            psm = ctx.enter_context(tc.tile_pool(name="psm", bufs=2, space="PSUM"))
            dram = ctx.enter_context(tc.tile_pool(name="dram", bufs=1, space="DRAM"))

            # ---------- constants & weight staging ----------
            ident = sbw.tile([128, 128], bf16)
            make_identity(nc, ident[:])
            iota_i = sbw.tile([128, NODES_C], i32)
            nc.gpsimd.iota(iota_i[:], pattern=[[1, NODES_C]], base=0,
                           channel_multiplier=0)
            iota_b = sbw.tile([128, NODES_C], bf16)
            nc.vector.tensor_copy(iota_b[:], iota_i[:])

            eaT = sbw.tile([13, E_PAD], bf16)
            nc.sync.dma_start(eaT[:], t_eaT)

            # per-edge-tile index/scatter tiles
            idxt, awt = [], []
            for t in range(ET):
                sl = slice(t * 128, (t + 1) * 128)
                it = sbw.tile([128, 1], i32, name=f"idx{t}")
                nc.sync.dma_start(it[:], t_src[sl, :])
                idxt.append(it)
                dt_ = sbw.tile([128, 1], f32, name=f"dst{t}")
                nc.sync.dma_start(dt_[:], t_dst[sl, :])
                we_ = sbw.tile([128, 1], f32, name=f"we{t}")
                nc.sync.dma_start(we_[:], t_we[sl, :])
                eq = sbk.tile([128, NODES_C], bf16, tag="eqtmp")
                nc.vector.tensor_tensor(
                    eq[:], iota_b[:], dt_[:, 0:1].to_broadcast([128, NODES_C]),
                    op=OP.is_equal)
                aw = sbw.tile([128, NODES_C], bf16, name=f"aw{t}")
                nc.vector.tensor_tensor(
                    aw[:], eq[:], we_[:, 0:1].to_broadcast([128, NODES_C]),
                    op=OP.mult)
                awt.append(aw)

            # pooling one-hot (scaled)
            pwt = []
            for t in range(2):
                sl = slice(t * 128, (t + 1) * 128)
                bt = sbw.tile([128, 1], f32, name=f"bat{t}")
                nc.sync.dma_start(bt[:], t_batch[sl, :])
                wp = sbw.tile([128, 1], f32, name=f"wp{t}")
                nc.sync.dma_start(wp[:], t_wpool[sl, :])
                eqp = sbk.tile([128, B], bf16, tag="eqtmp2")
                nc.vector.tensor_tensor(
                    eqp[:], iota_b[:, 0:B], bt[:, 0:1].to_broadcast([128, B]),
                    op=OP.is_equal)
                pw = sbw.tile([128, B], bf16, name=f"pw{t}")
                nc.vector.tensor_tensor(
                    pw[:], eqp[:], wp[:, 0:1].to_broadcast([128, B]),
                    op=OP.mult)
                pwt.append(pw)

            xT41 = sbw.tile([41, NODES_C], bf16)
            nc.sync.dma_start(xT41[:], t_xT41)
            xTmid = sbw.tile([6, NODES_C], bf16)
            nc.sync.dma_start(xTmid[:], t_xTmid)
            xTg = sbw.tile([55, NODES_C], bf16)
            nc.sync.dma_start(xTg[:], t_xTg)

            # weight staging, ordered by first use so the big sc-layer W2P
            # transfers don't delay the front-end / a11 start.
            # Row-split every weight at the boundaries its matmuls contract
            # over (matmul operands must start at a fresh tile, base part 0).
            SPLITS = {"cfc_wg": [32, 64], "cfc_wh": [32, 64],
                      "cfc_wt": [32, 64], "cfc_wout": [32],
                      "x22_w": [32, 64], "xm3_w": [96, 128]}
            ORDER = (["x2_w", "x2_b", "inter_wv", "inter_wo", "gg_wv",
                      "gg_wo", "ga_wv", "ga_wo", "cfc_wg", "cfc_wh",
                      "cfc_wt", "cfc_bg", "cfc_bh", "cfc_bt", "cfc_wout",
                      "x22_w", "x22_b"]
                     + [f"a11_{s}" for s in ("w1", "w2p", "root", "b")]
                     + [f"a21_{s}" for s in ("w1", "w2p", "root", "b")]
                     + ["xm3_w", "xm3_b"]
                     + [f"sc1_{s}" for s in ("w1", "w2p", "root", "b")]
                     + [f"sc2_{s}" for s in ("w1", "w2p", "root", "b")]
                     + ["fc1_w", "fc1_b", "fc2_w", "fc2_b"])
            assert set(ORDER) == set(wt.keys())
            W = {}
            for nm in ORDER:
                ap = wt[nm]
                p, f = ap.shape
                cuts = SPLITS.get(nm, [128] if p > 128 else [])
                bounds = [0] + list(cuts) + [p]
                tiles = []
                for bi in range(len(bounds) - 1):
                    r0, r1 = bounds[bi], bounds[bi + 1]
                    tw = sbw.tile([r1 - r0, f], ap.dtype, name=f"w_{nm}_{bi}")
                    nc.sync.dma_start(tw[:], ap[r0:r1, :])
                    tiles.append(tw)
                W[nm] = tiles

            # ---------- helpers ----------
            def mm_chain(psum_ap, parts, stop=True):
                """parts: list of (lhsT_ap, rhs_ap); accumulate into psum_ap."""
                n = len(parts)
                for i, (lh, rh) in enumerate(parts):
                    nc.tensor.matmul(out=psum_ap, lhsT=lh, rhs=rh,
                                     start=(i == 0), stop=(stop and i == n - 1))

            def act_evict(out_ap, psum_ap, func, bias=0.0):
                nc.scalar.activation(out_ap, psum_ap, func, bias=bias)

            def lin_T(rhs_parts, wname, M, func=AF.Copy, bias=None, name="lt"):
                """out[M, NODES_C] = func(sum_k w[k-chunk].T @ rhs_parts[k] + b).

                rhs_parts: list of SBUF tiles covering the contraction dim in
                chunks matching W[wname] row tiles; M = out channels (<=128).
                """
                ps = psm.tile([M, NODES_C], f32, name=f"ps_{name}", tag="mps")
                parts = []
                wtl = W[wname]
                assert len(wtl) == len(rhs_parts)
                for wti, rp in zip(wtl, rhs_parts):
                    parts.append((wti[:, 0:M], rp))
                mm_chain(ps[:], parts)
                ot = sbf.tile([M, NODES_C], bf16, name=name)
                if bias is not None:
                    act_evict(ot[:], ps[:], func, bias=bias)
                else:
                    act_evict(ot[:], ps[:], func)
                return ot

            def transpose_to(out_tile, out_col0, in_ap, w, name):
                """PE-transpose in_ap [p<=128, w<=128] -> out_tile[:, col0:col0+p]"""
                p = in_ap.shape[0]
                tp = psm.tile([w, 128], bf16, space="PSUM", name=f"tp_{name}",
                              tag="mps")
                nc.tensor.transpose(out=tp[:w, 0:p], in_=in_ap,
                                    identity=ident[0:p, 0:p])
                nc.vector.tensor_copy(out_tile[:, out_col0:out_col0 + p],
                                      tp[:w, 0:p])

            # ---------- front end (node-dense, transposed) ----------
            x2f = lin_T([xTmid[:]], "x2_w", FD, AF.Relu,
                        W["x2_b"][0][:, 0:1], name="x2f")
            p1 = lin_T([xTg[:]], "inter_wv", FD, name="p1")
            interT = lin_T([p1[:]], "inter_wo", FD, name="interT")
            p2 = lin_T([interT[:]], "gg_wv", FD, name="p2")
            globT = lin_T([p2[:]], "gg_wo", FD, name="globT")
            p3 = lin_T([globT[:]], "ga_wv", FD, name="p3")
            grpT = lin_T([p3[:]], "ga_wo", FD, name="grpT")

            # CfC, 3 unrolled steps; h kept as two [32, NODES_C] tiles
            uT = x2f
            h_lo, h_hi = grpT, globT
            steps = []
            for st in range(3):
                gates = {}
                for gname, wn, bn, fn in [("g1", "cfc_wg", "cfc_bg", AF.Tanh),
                                          ("g2", "cfc_wh", "cfc_bh", AF.Tanh),
                                          ("tt", "cfc_wt", "cfc_bt", AF.Sigmoid)]:
                    wk = W[wn]
                    btile = W[bn][0]
                    halves = []
                    for hf in range(2):
                        cs = slice(hf * FD, hf * FD + FD)
                        ps = psm.tile([FD, NODES_C], f32,
                                      name=f"cfc{st}{gname}{hf}", tag="mps")
                        mm_chain(ps[:], [(wk[0][:, cs], uT[:]),
                                         (wk[1][:, cs], h_lo[:]),
                                         (wk[2][:, cs], h_hi[:])])
                        g = sbf.tile([FD, NODES_C], bf16,
                                     name=f"{gname}{st}{hf}")
                        act_evict(g[:], ps[:], fn, bias=btile[cs, 0:1])
                        halves.append(g)
                    gates[gname] = halves
                new_h = []
                for hf in range(2):
                    g1, g2, tg = gates["g1"][hf], gates["g2"][hf], gates["tt"][hf]
                    d = sbk.tile([FD, NODES_C], bf16, tag="cfc_d")
                    nc.vector.tensor_tensor(d[:], g2[:], g1[:], op=OP.subtract)
                    td = sbk.tile([FD, NODES_C], bf16, tag="cfc_td")
                    nc.vector.tensor_tensor(td[:], tg[:], d[:], op=OP.mult)
                    hn = sbf.tile([FD, NODES_C], bf16, name=f"h{st}{hf}")
                    nc.vector.tensor_tensor(hn[:], g1[:], td[:], op=OP.add)
                    new_h.append(hn)
                h_lo, h_hi = new_h
                psu = psm.tile([FD, NODES_C], f32, name=f"cfcu{st}", tag="mps")
                mm_chain(psu[:], [(W["cfc_wout"][0][:], h_lo[:]),
                                  (W["cfc_wout"][1][:], h_hi[:])])
                uT = sbf.tile([FD, NODES_C], bf16, name=f"u{st}")
                act_evict(uT[:], psu[:], AF.Copy)
                steps.append(uT)

            # x2o = relu(cat(steps) @ x22_w + b)   [96, NODES_C]
            ps_x2o = psm.tile([96, NODES_C], f32, tag="mps")
            mm_chain(ps_x2o[:], [(W["x22_w"][k][:], steps[k][:])
                                 for k in range(3)])
            x2o = sbf.tile([96, NODES_C], bf16)
            act_evict(x2o[:], ps_x2o[:], AF.Relu, bias=W["x22_b"][0][:, 0:1])

            # ---------- conv layer runner ----------
            def conv_layer(L, table_ap, featT_parts, name):
                """Run NNConv L. table_ap: DRAM [N, din] bf16 row table.
                featT_parts: transposed own-node features (k-chunk tiles).
                Returns list of out tiles ([<=128, NODES_C]) covering dout."""
                din, dout = CONVS[L]["din"], CONVS[L]["dout"]
                kcs = _kchunks(din)
                ocs = _ochunks(dout)
                w2p = W[f"{L}_w2p"]
                w1 = W[f"{L}_w1"][0]
                root = W[f"{L}_root"]
                bias_tiles = W[f"{L}_b"]
                mcs = _kchunks(dout)  # out partition chunks

                s_ps = [pss.tile([m1 - m0, NODES_C], f32,
                                 name=f"sps_{name}{mi}", tag=f"sps{mi}")
                        for mi, (m0, m1) in enumerate(mcs)]

                n_acc = ET + len(kcs)  # matmuls accumulated per psum
                acc_i = [0] * len(mcs)

                for t in range(ET):
                    esl = slice(t * 128, (t + 1) * 128)
                    # edge MLP h
                    h_ps = psm.tile([128, FD], f32, tag="mps")
                    nc.tensor.matmul(out=h_ps[:], lhsT=eaT[:, esl], rhs=w1[:],
                                     start=True, stop=True)
                    hp = sbk.tile([128, FP], bf16, tag="hp")
                    act_evict(hp[:, 0:FD], h_ps[:], AF.Relu)
                    nc.vector.memset(hp[:, FD:FP], 1.0)
                    # gather + transpose
                    xr = sbk.tile([128, din], bf16, tag="xr")
                    nc.gpsimd.indirect_dma_start(
                        out=xr[:], out_offset=None, in_=table_ap,
                        in_offset=bass.IndirectOffsetOnAxis(
                            ap=idxt[t][:, 0:1], axis=0))
                    xsT = []
                    for ki, (k0, k1) in enumerate(kcs):
                        xt = sbk.tile([k1 - k0, 128], bf16, tag=f"xsT{ki}")
                        tp = psm.tile([k1 - k0, 128], bf16, space="PSUM",
                                      tag="mps")
                        nc.tensor.transpose(out=tp[:], in_=xr[:, k0:k1],
                                            identity=ident[:])
                        nc.vector.tensor_copy(xt[:], tp[:])
                        xsT.append(xt)
                    # Q chunks -> QH -> msg. Each o-chunk (<=30 o's) maps to
                    # two bank-aligned matmul windows of one 2-bank PSUM tile
                    # (a single matmul output may not cross a PSUM bank).
                    msg = sbk.tile([128, dout], bf16, tag="msg")
                    for (o0, o1) in ocs:
                        ow = o1 - o0
                        wins = [(0, min(15, ow))]
                        if ow > 15:
                            wins.append((15, ow))
                        q_ps = psq.tile([128, 1024], f32, tag="qps")
                        for wi, (a, b) in enumerate(wins):
                            for ki in range(len(kcs)):
                                nc.tensor.matmul(
                                    out=q_ps[:, wi * 512:wi * 512 + (b - a) * FP],
                                    lhsT=xsT[ki][:],
                                    rhs=w2p[ki][:, (o0 + a) * FP:(o0 + b) * FP],
                                    start=(ki == 0), stop=(ki == len(kcs) - 1))
                        qh = sbk.tile([128, ow * FP], bf16, tag="qh")
                        if ow == 30:
                            # one TT over both windows via a gapped 4D view
                            nc.vector.tensor_tensor(
                                qh[:].rearrange("p (b o f) -> p b o f",
                                                b=2, f=FP),
                                q_ps[:].rearrange("p (b x) -> p b x", b=2)
                                [:, :, 0:15 * FP].rearrange(
                                    "p b (o f) -> p b o f", f=FP),
                                hp[:, None, None, 0:FP].to_broadcast(
                                    [128, 2, 15, FP]),
                                op=OP.mult)
                        else:
                            for wi, (a, b) in enumerate(wins):
                                nc.vector.tensor_tensor(
                                    qh[:, a * FP:b * FP].rearrange(
                                        "p (o f) -> p o f", f=FP),
                                    q_ps[:, wi * 512:wi * 512 + (b - a) * FP]
                                    .rearrange("p (o f) -> p o f", f=FP),
                                    hp[:, None, 0:FP].to_broadcast(
                                        [128, b - a, FP]),
                                    op=OP.mult)
                        with nc.allow_low_precision("bf16 msg accum"):
                            nc.vector.tensor_reduce(
                                out=msg[:, o0:o1],
                                in_=qh[:].rearrange("p (o f) -> p o f", f=FP),
                                axis=mybir.AxisListType.X, op=OP.add)
                    # scatter
                    for mi, (m0, m1) in enumerate(mcs):
                        nc.tensor.matmul(out=s_ps[mi][:],
                                         lhsT=msg[:, m0:m1], rhs=awt[t][:],
                                         start=(acc_i[mi] == 0), stop=False)
                        acc_i[mi] += 1
                # root term
                outs = []
                for mi, (m0, m1) in enumerate(mcs):
                    for ki, (k0, k1) in enumerate(kcs):
                        acc_i[mi] += 1
                        nc.tensor.matmul(out=s_ps[mi][:],
                                         lhsT=root[ki][:, m0:m1],
                                         rhs=featT_parts[ki][:],
                                         start=False,
                                         stop=(acc_i[mi] == n_acc))
                    ot = sbf.tile([m1 - m0, NODES_C], bf16,
                                  name=f"conv_{name}{mi}")
                    act_evict(ot[:], s_ps[mi][:], AF.Relu,
                              bias=bias_tiles[mi][:, 0:1])
                    outs.append(ot)
                return outs

            def rows_and_allgather(featT_parts, D, name):
                """Transpose [feat,node] tiles into row tiles, DMA to DRAM,
                AllGather -> full [N, D] table."""
                slab = dram.tile([NODES_C, D], bf16, name=f"slab_{name}")
                tbl = dram.tile([N, D], bf16, addr_space="Shared",
                                name=f"tbl_{name}")
                for ntile in range(2):
                    nsl = slice(ntile * 128, (ntile + 1) * 128)
                    rows = sbk.tile([128, D], bf16, tag="rows")
                    col0 = 0
                    for ki, ft in enumerate(featT_parts):
                        w = ft.shape[0]
                        transpose_to(rows, col0, ft[:, nsl], 128,
                                     f"r{name}{ntile}{ki}")
                        col0 += w
                    nc.sync.dma_start(slab[nsl, :], rows[:])
                nc.gpsimd.collective_compute(
                    "AllGather", OP.bypass, ins=[slab.opt()], outs=[tbl.opt()],
                    replica_groups=RG)
                return tbl

            # ---------- a11 ----------
            x1 = conv_layer("a11", t_x41, [xT41[:]], "a11")[0]

            # ---------- a21 ----------
            tbl_a21 = rows_and_allgather([x2o], 96, "a21")
            x2o2 = conv_layer("a21", tbl_a21[:], [x2o[:]], "a21")[0]

            # ---------- xm ----------
            xm_parts = []
            for mi, (m0, m1) in enumerate(_kchunks(HD)):
                ps = psm.tile([m1 - m0, NODES_C], f32, name=f"psxm{mi}",
                              tag="mps")
                mm_chain(ps[:], [(W["xm3_w"][0][:, m0:m1], x2o2[:]),
                                 (W["xm3_w"][1][:, m0:m1], x1[:]),
                                 (W["xm3_w"][2][:, m0:m1], globT[:])])
                xm = sbf.tile([m1 - m0, NODES_C], bf16, name=f"xm{mi}")
                act_evict(xm[:], ps[:], AF.Relu,
                          bias=W["xm3_b"][mi][:, 0:1])
                xm_parts.append(xm)

            # ---------- sc1, sc2 ----------
            tbl_sc1 = rows_and_allgather(xm_parts, HD, "sc1")
            s1_parts = conv_layer("sc1", tbl_sc1[:], xm_parts, "sc1")
            tbl_sc2 = rows_and_allgather(s1_parts, HD, "sc2")
            s2_parts = conv_layer("sc2", tbl_sc2[:], s1_parts, "sc2")

            # ---------- pooling ----------
            pl_ps = pss.tile([B, HD], f32, tag="sps0")
            acc = 0
            for ntile in range(2):
                nsl = slice(ntile * 128, (ntile + 1) * 128)
                s2r = sbk.tile([128, HD], bf16, tag="rows")
                col0 = 0
                for ki, ft in enumerate(s2_parts):
                    w = ft.shape[0]
                    transpose_to(s2r, col0, ft[:, nsl], 128, f"pool{ntile}{ki}")
                    col0 += w
                nc.tensor.matmul(out=pl_ps[:], lhsT=pwt[ntile][:], rhs=s2r[:],
                                 start=(acc == 0), stop=(acc == 1))
                acc += 1
            pl_sb = sbf.tile([B, HD], f32)
            nc.vector.tensor_copy(pl_sb[:], pl_ps[:])
            ar_in = dram.tile([B, HD], f32)
            ar_out = dram.tile([B, HD], f32, addr_space="Shared")
            nc.sync.dma_start(ar_in[:], pl_sb[:])
            nc.gpsimd.collective_compute(
                "AllReduce", OP.add, ins=[ar_in.opt()], outs=[ar_out.opt()],
                replica_groups=RG)
            pooled = sbf.tile([B, HD], f32)
            nc.sync.dma_start(pooled[:], ar_out[:])

            # ---------- head ----------
            pooled_bf = sbf.tile([B, HD], bf16)
            nc.vector.tensor_copy(pooled_bf[:], pooled[:])
            pT_parts = []
            for ki, (k0, k1) in enumerate(_kchunks(HD)):
                tp = psm.tile([k1 - k0, B], bf16, space="PSUM",
                              name=f"ptp{ki}", tag="mps")
                nc.tensor.transpose(out=tp[:], in_=pooled_bf[:, k0:k1],
                                    identity=ident[0:B, 0:B])
                pt = sbf.tile([k1 - k0, B], bf16, name=f"pT{ki}")
                nc.vector.tensor_copy(pt[:], tp[:])
                pT_parts.append(pt)
            ps_t1 = psm.tile([128, B], f32, tag="mps")
            mm_chain(ps_t1[:], [(W["fc1_w"][0][:, :], pT_parts[0][:]),
                                (W["fc1_w"][1][:, :], pT_parts[1][:])])
            t1 = sbf.tile([128, B], bf16)
            act_evict(t1[:], ps_t1[:], AF.Relu, bias=W["fc1_b"][0][:, 0:1])
            ps_o = psm.tile([1, B], f32, tag="mps")
            nc.tensor.matmul(out=ps_o[:], lhsT=W["fc2_w"][0][:, :], rhs=t1[:],
                             start=True, stop=True)
            orow = sbf.tile([1, B], f32)
            act_evict(orow[:], ps_o[:], AF.Identity, bias=W["fc2_b"][0][:, 0:1])
            nc.sync.dma_start(t_out.rearrange("a b -> b a"), orow[:])

    nc.compile()
    return nc


_NC_CACHE = {}


def _get_program():
    if "nc" not in _NC_CACHE:
        _NC_CACHE["nc"] = build_program()
    return _NC_CACHE["nc"]


def _bf(x):
    import ml_dtypes
    return np.asarray(x, np.float32).astype(ml_dtypes.bfloat16)


def prepare_in_maps(x, edge_index, edge_attr, batch, params):
    x = np.asarray(x, np.float32)
    edge_index = np.asarray(edge_index, np.int64)
    edge_attr = np.asarray(edge_attr, np.float32)
    batch = np.asarray(batch, np.int64)
    p = {k: np.asarray(v, np.float32) for k, v in params.items()
         if not isinstance(v, dict)}
    for k, v in params.items():
        if isinstance(v, dict):
            p[k] = {kk: np.asarray(vv, np.float32) for kk, vv in v.items()}

    src, dst = edge_index[0], edge_index[1]
    cnt = np.bincount(dst, minlength=N).astype(np.float32)
    w_edge_all = 1.0 / np.maximum(cnt, 1.0)
    pcnt = np.bincount(batch, minlength=B).astype(np.float32)
    wpool_all = 1.0 / np.maximum(pcnt, 1.0)

    # shared (replicated) weight arrays
    shared = {"x41t": _bf(x[:, :41])}
    for L in CONVS:
        cfg = CONVS[L]
        din, dout = cfg["din"], cfg["dout"]
        pl = p[L]
        w1a = np.concatenate([pl["w1"], pl["b1"][None, :]], 0)  # [13, 32]
        shared[f"{L}_w1"] = _bf(w1a)
        w2r = pl["w2"].reshape(FD, din, dout)
        b2 = pl["b2"].reshape(din, dout)
        w2p = np.concatenate([w2r.transpose(1, 2, 0), b2[:, :, None]],
                             axis=2).reshape(din, dout * FP)
        shared[f"{L}_w2p"] = _bf(w2p)
        shared[f"{L}_root"] = _bf(pl["root"])
        shared[f"{L}_b"] = pl["bias"].reshape(dout, 1).astype(np.float32)
    for nm in ["x2_w", "inter_wv", "inter_wo", "gg_wv", "gg_wo", "ga_wv",
               "ga_wo", "cfc_wg", "cfc_wh", "cfc_wt", "cfc_wout", "x22_w",
               "xm3_w", "fc1_w", "fc2_w"]:
        shared[nm] = _bf(p[nm])
    for nm in ["x2_b", "cfc_bg", "cfc_bh", "cfc_bt", "x22_b", "xm3_b",
               "fc1_b", "fc2_b"]:
        shared[nm] = p[nm].reshape(-1, 1).astype(np.float32)

    in_maps = []
    owner = dst // NODES_C
    for c in range(NCORES):
        sel = np.nonzero(owner == c)[0]
        ec = len(sel)
        assert ec <= E_PAD, f"core {c} has {ec} edges > E_PAD={E_PAD}"
        srcc = np.zeros(E_PAD, np.int32)
        dstc = np.zeros(E_PAD, np.float32)
        wec = np.zeros(E_PAD, np.float32)
        eac = np.zeros((13, E_PAD), np.float32)
        srcc[:ec] = src[sel]
        dstc[:ec] = (dst[sel] - c * NODES_C).astype(np.float32)
        wec[:ec] = w_edge_all[dst[sel]]
        eac[:12, :ec] = edge_attr[sel].T
        eac[12, :ec] = 1.0
        nsl = slice(c * NODES_C, (c + 1) * NODES_C)
        m = dict(shared)
        xcT = x[nsl].T
        m["xT41"] = _bf(xcT[0:41])
        m["xTmid"] = _bf(xcT[41:47])
        m["xTg"] = _bf(xcT[160:215])
        m["eaT"] = _bf(eac)
        m["srcidx"] = srcc.reshape(E_PAD, 1)
        m["dstf"] = dstc.reshape(E_PAD, 1)
        m["wef"] = wec.reshape(E_PAD, 1)
        m["batchf"] = batch[nsl].astype(np.float32).reshape(NODES_C, 1)
        m["wpool"] = wpool_all[batch[nsl]].astype(np.float32).reshape(
            NODES_C, 1)
        in_maps.append(m)
    return in_maps


def kernel(x, edge_index, edge_attr, batch, params):
    in_maps = prepare_in_maps(x, edge_index, edge_attr, batch, params)
    from concourse.bass_utils import run_bass_kernel_spmd
    nc = _get_program()
    res = run_bass_kernel_spmd(nc, in_maps, core_ids=list(range(NCORES)))
    return np.asarray(res.results[0]["out"], np.float32)


if __name__ == "__main__":
    sys.path.insert(0, "/root/problem")
    import reference
    inputs = {k: np.asarray(v) for k, v in reference.setup_inputs().items()
              if not isinstance(v, dict)}
    inputs["params"] = {
        k: (np.asarray(v) if not isinstance(v, dict)
            else {kk: np.asarray(vv) for kk, vv in v.items()})
        for k, v in reference.setup_inputs()["params"].items()}
    out = kernel(**inputs)
    print(out[:8, 0])
